# revision 3
# baseline (speedup 1.0000x reference)
"""MetaOptNet SVM-CS head on 8 Trainium2 NeuronCores — compat-path version.

Math (unchanged from the converged baseline): the reference's 15-iteration
Mehrotra interior-point solve is fully converged, so we compute the QP
optimum directly per task:

    K  = S S^T                       (25x25 Gram, 4 tasks per 128-col window)
    W~ = (K + (1+rho) I)^{-1}        (2 Newton-Schulz iters, Chebyshev init)
    4x over-relaxed ADMM (rho=8, alpha=1.7) in compressed (D, sv) state
    logits = scale * Q S^T x         (x = last z / alpha)

Structural change vs the previous kernel: instead of w = S^T x (stage 4,
80 weight loads on sn) followed by logits = Q w (stage 5, 200 Q-window
weight loads — together ~22us of LDWEIGHTS-bound PE time and 2.6 MB of
extra DMA), we compute

    compat_g = stb_g^T qt_g          ([128 (tp,s), 300 (tp,q)] per group,
                                      bf16 stationary x fp8 moving, N=300)
    logits_g = xdiag_g^T compat_g    (4 col-tiled concurrent matmuls into
                                      one PSUM bank, 20-col weights)

which reuses the same 128-col stb weight windows as the Gram, needs no sn
tensor at all, and replaces 280 weight loads with 84 matmuls. Support ships
ONCE, in bf16 (stb) — this also upgrades the Gram from fp8 to bf16, paying
for the fp8 noise the compat path adds on S (sim: 1.53e-2 vs baseline's
1.58e-2, tolerance 2e-2).

Cross-task junk blocks: the full-window Gram matmul leaves junk in
cross-task blocks (masked before Newton-Schulz, as before); compat's
cross-task columns are junk too but land in logits rows/cols the host
never reads (xdiag is block-diagonal, pad rows are zero).

Sharding: pure data parallel, 16 tasks per core; host work is layout only.

DMA order: consts first (small), then stb (gates Gram -> NS -> ADMM, which
all hide under the qt stream), then qt group-by-group (compat tracks it).
Everything is a plain 2D [128, X] transfer with >=1.5KB contiguous
per-partition runs.
"""

import sys

sys.path.insert(0, "/opt/trn_rl_repo")

from contextlib import ExitStack

import numpy as np

import concourse.bass as bass
import concourse.tile as tile
from concourse import mybir
from concourse.alu_op_type import AluOpType
from concourse.bass_utils import run_bass_kernel_spmd
from concourse.tile import TileContext

# ---------------------------------------------------------------------------
# Problem constants (hardcoded per the harness contract)
N_CORES = 8
B_TOT = 128
T = 16            # tasks per core
NS = 25           # support samples per task
NW = 5            # ways
NQ = 75           # queries per task
D = 2560          # feature dim
NCH = D // 128    # 20 d-chunks
G = 4             # task groups per core (4 tasks each -> 128-col windows)
GP = T // G       # tasks per group
GQ = GP * NQ      # query cols per group (300)
SW = G * 128      # stb cols per chunk (512)
RHO = 8.0
NS_C = 0.0778     # Chebyshev-optimal NS init: 2/(eig_min+eig_max) of H
NS_ITERS = 2
ADMM_ITERS = 4    # over-relaxed (alpha) ADMM converges ~2.5x faster than plain
ALPHA = 1.7       # over-relaxation factor
C_REG = 0.1
QSCALE = 64.0     # fp8-e3m4 prescale for Q; descale folded into out scale

F32 = mybir.dt.float32
BF16 = mybir.dt.bfloat16
QDT = mybir.dt.float8e3


# ---------------------------------------------------------------------------
# The walrus build here encodes at most ONE sync-wait command per instruction
# (TPB_CTRL / S3_LW setupSyncWait raises "Too many sync wait commands").
# Tile's scheduler freely attaches several waits to one instruction, so after
# scheduling we split the excess onto NoOps inserted immediately before the
# instruction on the same engine — identical semantics, encodable waits.
def _split_waits(nc, max_waits=1):
    cnt = 0
    for blk in nc.m.functions[0].blocks:
        insns = blk.instructions
        idx = 0
        while idx < len(insns):
            ins = insns[idx]
            si = ins.sync_info
            waits = list(si.on_wait) if si and si.on_wait else []
            if len(waits) > max_waits:
                si.on_wait = waits[:max_waits]
                for w in waits[max_waits:]:
                    nop = mybir.InstNoOp(name=f"waitnop_{cnt}", ins=[], outs=[])
                    cnt += 1
                    nop.engine = ins.engine
                    nop.sync_info = mybir.SyncInfo(on_wait=[w], on_update=[])
                    nc.register_instruction(nop, overwrite=True)
                    insns.insert(idx, nop)
                    idx += 1
            idx += 1
    return cnt


# ---------------------------------------------------------------------------
def _build_program(repeat: int = 1, unroll: int = 1, variant: str = "full", upto: int = 5):
    """repeat>1 wraps the body in a hardware loop for slope timing (test.py).
    The graded kernel() path always uses repeat=1, unroll=1, variant="full".

    variant: "full" | "dma_only" (loads + store only) | "compute_only"
    (loads hoisted out of the loop) | "debug" (extra dumps)."""
    nc = bass.Bass("TRN2", target_bir_lowering=False)

    stb_d = nc.dram_tensor("stb", [128, NCH * SW], BF16, kind="ExternalInput")
    qtg_d = nc.dram_tensor("qtg", [G, 128, NCH * GQ], QDT, kind="ExternalInput")
    maskq_d = nc.dram_tensor("maskq", [128, 128], BF16, kind="ExternalInput")
    nine_d = nc.dram_tensor("nine", [128, 128], F32, kind="ExternalInput")
    i2_d = nc.dram_tensor("i2", [128, 128], F32, kind="ExternalInput")
    cib_d = nc.dram_tensor("cib", [128, 128], BF16, kind="ExternalInput")
    ohc_d = nc.dram_tensor("ohc", [128, 20], F32, kind="ExternalInput")
    h2_d = nc.dram_tensor("h2", [128, 20], F32, kind="ExternalInput")
    hmo_d = nc.dram_tensor("hmo", [128, 20], F32, kind="ExternalInput")
    scale_d = nc.dram_tensor("scale", [1, 1], F32, kind="ExternalInput")
    out_d = nc.dram_tensor("out", [G * 20, GQ], F32, kind="ExternalOutput")
    if variant == "debug":
        dbg_h = nc.dram_tensor("dbg_h", [128, 128], F32, kind="ExternalOutput")
        dbg_wt = nc.dram_tensor("dbg_wt", [128, 128], F32, kind="ExternalOutput")
        dbg_xb = nc.dram_tensor("dbg_xb", [128, 20], BF16, kind="ExternalOutput")
        dbg_cs = nc.dram_tensor("dbg_cs", [128, GQ], BF16, kind="ExternalOutput")

    with ExitStack() as ctx:
        tc = ctx.enter_context(TileContext(nc))
        stb_pool = ctx.enter_context(tc.tile_pool(name="stb", bufs=1))
        qt_pool = ctx.enter_context(tc.tile_pool(name="qt", bufs=G))
        consts = ctx.enter_context(tc.tile_pool(name="consts", bufs=1))
        mats = ctx.enter_context(tc.tile_pool(name="mats", bufs=12))
        state = ctx.enter_context(tc.tile_pool(name="state", bufs=14))
        csb_pool = ctx.enter_context(tc.tile_pool(name="csb", bufs=G))

        def emit_loads():
            # small consts first (gate NS/ADMM, land well before needed)
            maskq_sb = consts.tile([128, 128], BF16, tag="maskq")
            nc.scalar.dma_start(out=maskq_sb, in_=maskq_d[:, :])
            nine_sb = consts.tile([128, 128], F32, tag="nine")
            nc.sync.dma_start(out=nine_sb, in_=nine_d[:, :])
            i2_sb = consts.tile([128, 128], F32, tag="i2")
            nc.scalar.dma_start(out=i2_sb, in_=i2_d[:, :])
            cib_sb = consts.tile([128, 128], BF16, tag="cib")
            nc.sync.dma_start(out=cib_sb, in_=cib_d[:, :])
            ohc_sb = consts.tile([128, 20], F32, tag="ohc")
            nc.scalar.dma_start(out=ohc_sb, in_=ohc_d[:, :])
            h2_sb = consts.tile([128, 20], F32, tag="h2")
            nc.scalar.dma_start(out=h2_sb, in_=h2_d[:, :])
            hmo_sb = consts.tile([128, 20], F32, tag="hmo")
            nc.sync.dma_start(out=hmo_sb, in_=hmo_d[:, :])
            scale_sb = consts.tile([128, 1], F32, tag="scale")
            nc.sync.dma_start(out=scale_sb, in_=scale_d[:, :].to_broadcast([128, 1]))

            # ADMM state init: d1 = ohc (bf16), s = hmo/alpha
            d1f_sb = state.tile([128, 20], F32, tag="d1f")
            nc.scalar.dma_start(out=d1f_sb, in_=ohc_d[:, :])
            d1_sb = state.tile([128, 20], BF16, tag="d1")
            nc.vector.tensor_copy(d1_sb, d1f_sb)
            s0_sb = state.tile([128, 20], F32, tag="s0")
            nc.scalar.activation(
                s0_sb, hmo_sb, mybir.ActivationFunctionType.Copy, scale=1.0 / ALPHA
            )

            # stb: 10 transfers of 2 chunks each ([128, 1024] bf16, 2KB/part)
            stb_tile = stb_pool.tile([128, NCH * SW], BF16, tag="stb")
            for j in range(10):
                eng = nc.sync if j % 2 == 0 else nc.scalar
                eng.dma_start(
                    out=stb_tile[:, j * 2 * SW : (j + 1) * 2 * SW],
                    in_=stb_d[:, j * 2 * SW : (j + 1) * 2 * SW],
                )

            # qt: per group, 4 transfers of 5 chunks ([128, 1500] fp8)
            qt_sb = []
            for g in range(G):
                t_ = qt_pool.tile([128, NCH * GQ], QDT, tag="qt")
                for h in range(4):
                    eng = nc.sync if (g * 4 + h) % 2 == 0 else nc.scalar
                    eng.dma_start(
                        out=t_[:, h * 5 * GQ : (h + 1) * 5 * GQ],
                        in_=qtg_d[g, :, h * 5 * GQ : (h + 1) * 5 * GQ],
                    )
                qt_sb.append(t_)

            return dict(maskq=maskq_sb, nine=nine_sb, i2=i2_sb, cib=cib_sb,
                        ohc=ohc_sb, h2=h2_sb, hmo=hmo_sb, scale=scale_sb,
                        d1=d1_sb, s0=s0_sb, stb=stb_tile, qt=qt_sb)

        def emit_compute(hd):
            def early_out():
                zt = consts.tile([128, GQ], F32, tag="outsb")
                nc.vector.memset(zt, 0.0)
                nc.sync.dma_start(out=out_d[:, :], in_=zt[: G * 20, :])

            stb = hd["stb"]
            qt_sb = hd["qt"]
            maskq_sb, nine_sb, i2_sb, cib_sb = (
                hd["maskq"], hd["nine"], hd["i2"], hd["cib"])
            h2_sb, hmo_sb, scale_sb = hd["h2"], hd["hmo"], hd["scale"]
            d1_sb = hd["d1"]

            def stw(c, g):
                return stb[:, c * SW + g * 128 : c * SW + (g + 1) * 128]

            # ---- stage 1: K = S S^T per 4-task window, bf16 ---------------
            # chunk-outer so the matmuls track the stb DMA stream; 4 banks.
            hb_all = []
            with tc.tile_pool(name="kpsum", bufs=4, space="PSUM") as kpsum:
                kp_all = []
                for g in range(G):
                    kp = kpsum.tile([128, 128], F32, tag="kp")
                    kp_all.append(kp)
                for c in range(NCH):
                    for g in range(G):
                        nc.tensor.matmul(
                            kp_all[g], lhsT=stw(c, g), rhs=stw(c, g),
                            start=(c == 0), stop=(c == NCH - 1),
                        )
                for g in range(G):
                    km = mats.tile([128, 128], F32, tag="km")
                    nc.vector.tensor_tensor(km, kp_all[g], maskq_sb, op=AluOpType.mult)
                    hb = mats.tile([128, 128], BF16, tag="hb")
                    nc.vector.tensor_tensor(hb, km, nine_sb, op=AluOpType.add)
                    hb_all.append(hb)

            if upto < 2:
                return early_out()
            # ---- stage 2: Newton-Schulz inverse, all-bf16 -----------------
            wt_sb = []
            with tc.tile_pool(name="npsum", bufs=4, space="PSUM") as npsum:
                x_cur = [cib_sb] * G
                for it in range(NS_ITERS):
                    last = it == NS_ITERS - 1
                    for g in range(G):
                        t1p = npsum.tile([128, 128], F32, tag="t1p")
                        nc.tensor.matmul(
                            t1p, lhsT=hb_all[g], rhs=x_cur[g], start=True, stop=True
                        )
                        u_ns = mats.tile([128, 128], BF16, tag="u_nsb")
                        nc.vector.tensor_tensor(u_ns, i2_sb, t1p, op=AluOpType.subtract)
                        x2p = npsum.tile([128, 128], F32, tag="x2p")
                        nc.tensor.matmul(
                            x2p, lhsT=x_cur[g], rhs=u_ns, start=True, stop=True
                        )
                        if last:
                            wt = mats.tile([128, 128], BF16, tag="wt")
                            nc.vector.tensor_copy(wt, x2p)
                            wt_sb.append(wt)
                        else:
                            x_next = mats.tile([128, 128], BF16, tag="x_nsb")
                            nc.vector.tensor_copy(x_next, x2p)
                            x_cur[g] = x_next

            if upto < 3:
                return early_out()
            # ---- stage 3: over-relaxed ADMM (compressed state) ------------
            #   z   = center(W (v - u + ohc));  zh = a*z + (1-a)*v
            #   r   = zh + u - h;  v' = h + (r - |r|)/2;  u' = relu(r)
            #   d1' = (h + ohc) - |r|  [consts pre-scaled by alpha*rho]
            xb_sb = None
            with tc.tile_pool(name="mpsum", bufs=2, space="PSUM") as mpsum:
                s_sb = hd["s0"]
                for it in range(ADMM_ITERS):
                    last = it == ADMM_ITERS - 1
                    xp = mpsum.tile([128, 20], F32, tag="mp")
                    for g in range(G):
                        nc.tensor.matmul(
                            xp[:, g * NW : (g + 1) * NW],
                            lhsT=wt_sb[g],
                            rhs=d1_sb[:, g * NW : (g + 1) * NW],
                            start=True,
                            stop=True,
                        )
                    msum = state.tile([128, 4], F32, tag="msum")
                    nc.vector.reduce_sum(
                        msum,
                        xp[:, :].rearrange("p (g w) -> p g w", w=NW),
                        axis=mybir.AxisListType.X,
                    )
                    msb = msum[:, :]
                    msb_ap = bass.AP(
                        tensor=msb.tensor, offset=msb.offset,
                        ap=[msb.ap[0], msb.ap[1], [0, NW]],
                    )
                    zn_sb = state.tile([128, 20], BF16 if last else F32,
                                       tag="xb" if last else "zn")
                    nc.vector.scalar_tensor_tensor(
                        out=zn_sb[:, :].rearrange("p (g w) -> p g w", w=NW),
                        in0=msb_ap,
                        scalar=-1.0 / NW,
                        in1=xp[:, :].rearrange("p (g w) -> p g w", w=NW),
                        op0=AluOpType.mult,
                        op1=AluOpType.add,
                    )
                    if last:
                        xb_sb = zn_sb
                        break
                    r_sb = state.tile([128, 20], F32, tag="r")
                    nc.vector.tensor_tensor(r_sb, zn_sb, s_sb, op=AluOpType.subtract)
                    a_sb = state.tile([128, 20], F32, tag="absr")
                    nc.vector.scalar_tensor_tensor(
                        out=a_sb, in0=r_sb, scalar=-1.0, in1=r_sb,
                        op0=AluOpType.mult, op1=AluOpType.max,
                    )
                    d1_sb = state.tile([128, 20], BF16, tag="d1n")
                    nc.vector.scalar_tensor_tensor(
                        out=d1_sb, in0=a_sb, scalar=-(RHO * ALPHA), in1=h2_sb,
                        op0=AluOpType.mult, op1=AluOpType.add,
                    )
                    # off-critical-path state maintenance:
                    m1_sb = state.tile([128, 20], F32, tag="m1")
                    nc.vector.scalar_tensor_tensor(
                        out=m1_sb, in0=r_sb, scalar=(1.0 - ALPHA), in1=r_sb,
                        op0=AluOpType.mult, op1=AluOpType.max,
                    )
                    s_sb = state.tile([128, 20], F32, tag="sst")
                    nc.vector.tensor_tensor(s_sb, hmo_sb, m1_sb, op=AluOpType.subtract)

            # xdiag: block-diagonal solution tiles, scale folded in.
            # xdiag[tp*32+s, tp*5+w] = xb[tp*32+s, g*5+w] * scale
            xdiag_all = []
            for g in range(G):
                xdg = state.tile([128, 20], BF16, tag=f"xdiag{g}")
                nc.vector.memset(xdg, 0.0)
                xdiag_all.append(xdg)
            for g in range(G):
                for tp in range(GP):
                    sl = slice(tp * 32, tp * 32 + NS)
                    ssb = scale_sb[sl, :]
                    sc_ap = bass.AP(
                        tensor=ssb.tensor, offset=ssb.offset,
                        ap=[ssb.ap[0], [0, NW]],
                    )
                    nc.vector.tensor_tensor(
                        xdiag_all[g][sl, tp * NW : (tp + 1) * NW],
                        xb_sb[sl, g * NW : (g + 1) * NW],
                        sc_ap,
                        op=AluOpType.mult,
                    )

            if upto < 4:
                return early_out()
            # ---- stage 4: compat_g = stb_g^T qt_g (bf16 x fp8, N=300) -----
            # group-outer / chunk-inner: tracks the group-serial qt stream.
            cs_all = []
            with tc.tile_pool(name="cpsum", bufs=4, space="PSUM") as cpsum:
                cp_all = []
                for g in range(G):
                    cp = cpsum.tile([128, GQ], F32, tag="cp")
                    for c in range(NCH):
                        nc.tensor.matmul(
                            cp,
                            lhsT=stw(c, g),
                            rhs=qt_sb[g][:, c * GQ : (c + 1) * GQ],
                            start=(c == 0),
                            stop=(c == NCH - 1),
                        )
                    cp_all.append(cp)
                    cs = csb_pool.tile([128, GQ], BF16, tag="cs")
                    nc.vector.tensor_copy(cs, cp)
                    cs_all.append(cs)

                if upto < 5:
                    return early_out()
                # ---- stage 5: logits_g = xdiag_g^T compat_g ---------------
                # 4 col-tiled concurrent matmuls into one PSUM bank; strips
                # at partitions 32g..32g+20. Copy strips to SBUF (DVE can't
                # shift partitions), then 4 strip DMAs to out rows 20g.
                with tc.tile_pool(name="lpsum", bufs=1, space="PSUM") as lpsum:
                    lp = lpsum.tile([128, GQ], F32, tag="lp")
                    for g in range(G):
                        nc.tensor.matmul(
                            lp[32 * g : 32 * g + 20, :],
                            lhsT=xdiag_all[g],
                            rhs=cs_all[g],
                            start=True,
                            stop=True,
                            tile_position=(0, 32 * g),
                        )
                    out_sb = consts.tile([128, GQ], F32, tag="outsb")
                    for g in range(G):
                        sl = slice(32 * g, 32 * g + 20)
                        nc.vector.tensor_copy(out_sb[sl, :], lp[sl, :])
                        eng = nc.sync if g % 2 == 0 else nc.scalar
                        eng.dma_start(
                            out=out_d[g * 20 : (g + 1) * 20, :],
                            in_=out_sb[sl, :],
                        )
            if variant == "debug":
                nc.sync.dma_start(out=dbg_h[:, :], in_=hb_all[0])
                dwt = mats.tile([128, 128], F32, tag="dbgwt")
                nc.vector.tensor_copy(dwt, wt_sb[0])
                nc.sync.dma_start(out=dbg_wt[:, :], in_=dwt)
                nc.sync.dma_start(out=dbg_xb[:, :], in_=xb_sb)
                nc.sync.dma_start(out=dbg_cs[:, :], in_=cs_all[0])

        def emit_body():
            if variant == "dma_only":
                hd = emit_loads()
                zt = consts.tile([128, GQ], F32, tag="outsb")
                nc.vector.memset(zt, 0.0)
                nc.sync.dma_start(out=out_d[:, :], in_=zt[: G * 20, :])
            else:
                hd = emit_loads()
                emit_compute(hd)

        if variant == "compute_only":
            hd0 = emit_loads()
        if repeat > 1:
            try:
                ctx.enter_context(tc.For_i(0, repeat, 1, staggered_reset=True))
            except Exception:
                ctx.enter_context(tc.For_i(0, repeat, 1))
        for _ in range(unroll):
            if variant == "compute_only":
                emit_compute(hd0)
            else:
                emit_body()

    _split_waits(nc)
    return nc


_NC_CACHE = None


def _get_nc():
    global _NC_CACHE
    if _NC_CACHE is None:
        _NC_CACHE = _build_program()
    return _NC_CACHE


# ---------------------------------------------------------------------------
def _host_prep(support, query, support_labels, scale):
    """Shard + pack into the DMA layouts. Layout only, no FLOPs."""
    f32 = np.float32
    bf = mybir.dt.np(BF16)
    e3 = mybir.dt.np(QDT)
    eyebd = np.zeros((128, 128), dtype=f32)     # eye-25 block diagonal
    onesbd = np.zeros((128, 128), dtype=f32)    # ones 25x25 block diagonal
    for tp in range(GP):
        sl = slice(tp * 32, tp * 32 + NS)
        eyebd[sl, sl] = np.eye(NS, dtype=f32)
        onesbd[sl, sl] = 1.0
    maskq = np.ascontiguousarray(onesbd.astype(bf))
    nine = np.ascontiguousarray((1.0 + RHO) * eyebd)
    i2 = np.ascontiguousarray(2.0 * eyebd)
    cib = np.ascontiguousarray((NS_C * eyebd).astype(bf))
    sc = np.asarray(scale, dtype=f32).reshape(1, 1) / (QSCALE * ALPHA)

    in_maps = []
    for core in range(N_CORES):
        sl = slice(core * T, (core + 1) * T)
        S = np.asarray(support[sl], dtype=f32)        # [16,25,2560]
        Q = np.asarray(query[sl], dtype=f32)          # [16,75,2560]
        lab = np.asarray(support_labels[sl])          # [16,25] int
        # stb[p, c*512 + g*128 + tp*32 + s] = S[4g+tp, s, 128c+p]
        src = S.reshape(G, GP, NS, NCH, 128).transpose(4, 3, 0, 1, 2)
        arr = np.zeros((128, NCH, G, GP, 32), dtype=bf)
        arr[..., :NS] = src.astype(bf)
        stb = np.ascontiguousarray(arr.reshape(128, NCH * SW))
        # qtg[g, p, c*300 + tp*75 + q] = clip(64*Q)[4g+tp, q, 128c+p]
        q8 = np.clip(Q * QSCALE, -15.5, 15.5)
        qsrc = q8.reshape(G, GP, NQ, NCH, 128).transpose(0, 4, 3, 1, 2)
        qtg = np.ascontiguousarray(
            qsrc.reshape(G, 128, NCH * GQ).astype(e3)
        )
        oh = (lab[:, :, None] == np.arange(NW)[None, None, :]).astype(f32)
        # [16,25,5] -> [128,20]: row = tp*32+s, col = g*5+w
        ohm = np.zeros((128, 20), dtype=f32)
        ohr = oh.reshape(G, GP, NS, NW).transpose(1, 2, 0, 3).reshape(GP, NS, 20)
        for tp in range(GP):
            ohm[tp * 32 : tp * 32 + NS, :] = ohr[tp]
        in_maps.append(
            {
                "stb": stb,
                "qtg": qtg,
                "maskq": maskq,
                "nine": nine,
                "i2": i2,
                "cib": cib,
                "ohc": np.ascontiguousarray(ALPHA * ohm),
                "h2": np.ascontiguousarray(RHO * ALPHA * (C_REG + 1.0 / RHO) * ohm),
                "hmo": np.ascontiguousarray(ALPHA * C_REG * ohm),
                "scale": sc,
            }
        )
    return in_maps


def _unshard_out(o):
    """[80, GQ] -> [T, NQ, NW]. o[20g + 5tp + w, 75tp' + q] valid at tp==tp'."""
    r = np.asarray(o, np.float32).reshape(G, GP, NW, GP, NQ)
    d = np.einsum('gtwtq->gtqw', r)              # diagonal over tp
    return d.reshape(T, NQ, NW)


def kernel(query, support, scale, support_labels, n_way, n_shot):
    assert int(n_way) == NW and int(n_shot) * int(n_way) == NS
    assert query.shape == (B_TOT, NQ, D) and support.shape == (B_TOT, NS, D)
    nc = _get_nc()
    in_maps = _host_prep(support, query, support_labels, scale)
    res = run_bass_kernel_spmd(nc, in_maps, core_ids=list(range(N_CORES)))
    outs = []
    for core in range(N_CORES):
        o = np.asarray(res.results[core]["out"])      # [80, GQ]
        outs.append(_unshard_out(o))
    return np.ascontiguousarray(np.concatenate(outs, axis=0), dtype=np.float32)


# revision 12
# speedup vs baseline: 1.2641x; 1.2641x over previous
"""MetaOptNet SVM-CS head on 8 Trainium2 NeuronCores — compat-path version.

Math (unchanged from the converged baseline): the reference's 15-iteration
Mehrotra interior-point solve is fully converged, so we compute the QP
optimum directly per task:

    K  = S S^T                       (25x25 Gram, 4 tasks per 128-col window)
    W~ = (K + (1+rho) I)^{-1}        (2 Newton-Schulz iters, Chebyshev init)
    4x over-relaxed ADMM (rho=8, alpha=1.7) in compressed (D, sv) state
    logits = scale * Q S^T x         (x = last z / alpha)

Structural change vs the previous kernel: instead of w = S^T x (stage 4,
80 weight loads on sn) followed by logits = Q w (stage 5, 200 Q-window
weight loads — together ~22us of LDWEIGHTS-bound PE time and 2.6 MB of
extra DMA), we compute

    compat_g = stb_g^T qt_g          ([128 (tp,s), 300 (tp,q)] per group,
                                      bf16 stationary x fp8 moving, N=300)
    logits_g = xdiag_g^T compat_g    (4 col-tiled concurrent matmuls into
                                      one PSUM bank, 20-col weights)

which reuses the same 128-col stb weight windows as the Gram, needs no sn
tensor at all, and replaces 280 weight loads with 84 matmuls. Support ships
ONCE, in bf16 (stb) — this also upgrades the Gram from fp8 to bf16, paying
for the fp8 noise the compat path adds on S (sim: 1.53e-2 vs baseline's
1.58e-2, tolerance 2e-2).

Cross-task junk blocks: the full-window Gram matmul leaves junk in
cross-task blocks (masked before Newton-Schulz, as before); compat's
cross-task columns are junk too but land in logits rows/cols the host
never reads (xdiag is block-diagonal, pad rows are zero).

Sharding: pure data parallel, 16 tasks per core; host work is layout only.

DMA order: consts first (small), then stb (gates Gram -> NS -> ADMM, which
all hide under the qt stream), then qt group-by-group (compat tracks it).
Everything is a plain 2D [128, X] transfer with >=1.5KB contiguous
per-partition runs.
"""

import sys

sys.path.insert(0, "/opt/trn_rl_repo")

from contextlib import ExitStack

import numpy as np

import concourse.bass as bass
import concourse.tile as tile
from concourse import mybir
from concourse.alu_op_type import AluOpType
from concourse.bass_utils import run_bass_kernel_spmd
from concourse.tile import TileContext

# ---------------------------------------------------------------------------
# Problem constants (hardcoded per the harness contract)
N_CORES = 8
B_TOT = 128
T = 16            # tasks per core
NS = 25           # support samples per task
NW = 5            # ways
NQ = 75           # queries per task
D = 2560          # feature dim
NCH = D // 128    # 20 d-chunks
G = 4             # task groups per core (4 tasks each -> 128-col windows)
GP = T // G       # tasks per group
GQ = GP * NQ      # query cols per group (300)
SW = G * 128      # stb cols per chunk (512)
RHO = 8.0
NS_C = 0.0778     # Chebyshev-optimal NS init: 2/(eig_min+eig_max) of H
NS_ITERS = 2
ADMM_ITERS = 4    # over-relaxed (alpha) ADMM converges ~2.5x faster than plain
ALPHA = 1.7       # over-relaxation factor
C_REG = 0.1
QSCALE = 64.0     # fp8-e3m4 prescale for Q; descale folded into out scale

F32 = mybir.dt.float32
BF16 = mybir.dt.bfloat16
QDT = mybir.dt.float8e3


# ---------------------------------------------------------------------------
# The walrus build here encodes at most ONE sync-wait command per instruction
# (TPB_CTRL / S3_LW setupSyncWait raises "Too many sync wait commands").
# Tile's scheduler freely attaches several waits to one instruction, so after
# scheduling we split the excess onto NoOps inserted immediately before the
# instruction on the same engine — identical semantics, encodable waits.
def _split_waits(nc, max_waits=1):
    cnt = 0
    for blk in nc.m.functions[0].blocks:
        insns = blk.instructions
        idx = 0
        while idx < len(insns):
            ins = insns[idx]
            si = ins.sync_info
            waits = list(si.on_wait) if si and si.on_wait else []
            if len(waits) > max_waits:
                si.on_wait = waits[:max_waits]
                for w in waits[max_waits:]:
                    nop = mybir.InstNoOp(name=f"waitnop_{cnt}", ins=[], outs=[])
                    cnt += 1
                    nop.engine = ins.engine
                    nop.sync_info = mybir.SyncInfo(on_wait=[w], on_update=[])
                    nc.register_instruction(nop, overwrite=True)
                    insns.insert(idx, nop)
                    idx += 1
            idx += 1
    return cnt


# ---------------------------------------------------------------------------
def _build_program(repeat: int = 1, unroll: int = 1, variant: str = "full", upto: int = 5):
    """repeat>1 wraps the body in a hardware loop for slope timing (test.py).
    The graded kernel() path always uses repeat=1, unroll=1, variant="full".

    variant: "full" | "dma_only" (loads + store only) | "compute_only"
    (loads hoisted out of the loop) | "debug" (extra dumps)."""
    nc = bass.Bass("TRN2", target_bir_lowering=False)

    stb_d = nc.dram_tensor("stb", [128, NCH * G * NS], BF16, kind="ExternalInput")
    qtg_d = nc.dram_tensor("qtg", [G, 128, NCH * GQ], QDT, kind="ExternalInput")
    maskq_d = nc.dram_tensor("maskq", [128, 128], BF16, kind="ExternalInput")
    nine_d = nc.dram_tensor("nine", [128, 128], F32, kind="ExternalInput")
    i2_d = nc.dram_tensor("i2", [128, 128], F32, kind="ExternalInput")
    cib_d = nc.dram_tensor("cib", [128, 128], BF16, kind="ExternalInput")
    ohc_d = nc.dram_tensor("ohc", [128, 20], F32, kind="ExternalInput")
    h2_d = nc.dram_tensor("h2", [128, 20], F32, kind="ExternalInput")
    hmo_d = nc.dram_tensor("hmo", [128, 20], F32, kind="ExternalInput")
    scale_d = nc.dram_tensor("scale", [1, 1], F32, kind="ExternalInput")
    out_d = nc.dram_tensor("out", [G * 20, GQ], F32, kind="ExternalOutput")
    if variant == "debug":
        dbg_h = nc.dram_tensor("dbg_h", [128, 128], F32, kind="ExternalOutput")
        dbg_wt = nc.dram_tensor("dbg_wt", [128, 128], F32, kind="ExternalOutput")
        dbg_xb = nc.dram_tensor("dbg_xb", [128, 20], BF16, kind="ExternalOutput")
        dbg_cs = nc.dram_tensor("dbg_cs", [128, GQ], BF16, kind="ExternalOutput")

    with ExitStack() as ctx:
        tc = ctx.enter_context(TileContext(nc))
        stb_pool = ctx.enter_context(tc.tile_pool(name="stb", bufs=1))
        qt_pool = ctx.enter_context(tc.tile_pool(name="qt", bufs=G))
        consts = ctx.enter_context(tc.tile_pool(name="consts", bufs=1))
        mats = ctx.enter_context(tc.tile_pool(name="mats", bufs=12))
        state = ctx.enter_context(tc.tile_pool(name="state", bufs=14))
        csb_pool = ctx.enter_context(tc.tile_pool(name="csb", bufs=G))

        def emit_loads():
            # DMA order (both queues drain in emission order; ~300 GB/s
            # aggregate): consts -> stb chunks 0-9 -> qt g0 -> stb chunks
            # 10-19 -> qt g1..g3. qt-g0 rides between the stb halves so
            # compat-g0 matmuls can fill PE gaps in the gram/NS phase.
            _q = [0]

            def dma(out, in_):
                eng = nc.sync if _q[0] % 2 == 0 else nc.scalar
                _q[0] += 1
                eng.dma_start(out=out, in_=in_)

            # consts after stb-h1 (they gate NS/ADMM, needed ~mid-kernel)
            hd = {}

            def emit_consts():
                maskq_sb = consts.tile([128, 128], BF16, tag="maskq")
                dma(maskq_sb, maskq_d[:, :])
                nine_sb = consts.tile([128, 128], F32, tag="nine")
                dma(nine_sb, nine_d[:, :])
                i2_sb = consts.tile([128, 128], F32, tag="i2")
                dma(i2_sb, i2_d[:, :])
                cib_sb = consts.tile([128, 128], BF16, tag="cib")
                dma(cib_sb, cib_d[:, :])
                ohc_sb = consts.tile([128, 20], F32, tag="ohc")
                dma(ohc_sb, ohc_d[:, :])
                h2_sb = consts.tile([128, 20], F32, tag="h2")
                dma(h2_sb, h2_d[:, :])
                hmo_sb = consts.tile([128, 20], F32, tag="hmo")
                dma(hmo_sb, hmo_d[:, :])
                scale_sb = consts.tile([128, 1], F32, tag="scale")
                dma(scale_sb, scale_d[:, :].to_broadcast([128, 1]))
                d1f_sb = state.tile([128, 20], F32, tag="d1f")
                dma(d1f_sb, ohc_d[:, :])
                # ADMM state init: d1 = ohc (bf16), s = hmo/alpha
                d1_sb = state.tile([128, 20], BF16, tag="d1")
                nc.vector.tensor_copy(d1_sb, d1f_sb)
                s0_sb = state.tile([128, 20], F32, tag="s0")
                nc.scalar.activation(
                    s0_sb, hmo_sb, mybir.ActivationFunctionType.Copy,
                    scale=1.0 / ALPHA,
                )
                hd.update(maskq=maskq_sb, nine=nine_sb, i2=i2_sb, cib=cib_sb,
                          ohc=ohc_sb, h2=h2_sb, hmo=hmo_sb, scale=scale_sb,
                          d1=d1_sb, s0=s0_sb)

            stb_tile = stb_pool.tile([128, NCH * SW], BF16, tag="stb")
            stbt_tile = stb_pool.tile([128, NCH * G * NS], BF16, tag="stbt")
            qt_sb = []
            for g in range(G):
                t_ = qt_pool.tile([128, NCH * GQ], QDT, tag="qt")
                qt_sb.append(t_)

            # zero the pad cols (25-31 of each 32-strip) once up front: junk
            # there would poison compat rows / masked Gram blocks with NaNs.
            sfull = stb_tile[:, :]
            pad_ap = bass.AP(
                tensor=sfull.tensor, offset=sfull.offset + NS,
                ap=[sfull.ap[0], [SW, NCH], [128, G], [32, GP], [1, 32 - NS]],
            )
            nc.vector.memset(pad_ap, 0.0)

            TCOL = G * NS  # 100 tight cols per chunk

            def stb_half(h):
                # tight DMA ([128, 2*100] bf16 per 2-chunk transfer) + DVE
                # expand into the 32-stride window layout the weights need
                for j in range(5 * h, 5 * (h + 1)):
                    dma(
                        stbt_tile[:, j * 2 * TCOL : (j + 1) * 2 * TCOL],
                        stb_d[:, j * 2 * TCOL : (j + 1) * 2 * TCOL],
                    )
                    st_src = stbt_tile[:, j * 2 * TCOL : (j + 1) * 2 * TCOL]
                    src_ap = st_src.rearrange(
                        "p (k g t s) -> p k g t s", k=2, g=G, t=GP
                    )
                    sdst = stb_tile[:, :]
                    dst_ap = bass.AP(
                        tensor=sdst.tensor, offset=sdst.offset + j * 2 * SW,
                        ap=[sdst.ap[0], [SW, 2], [128, G], [32, GP], [1, NS]],
                    )
                    nc.vector.tensor_copy(dst_ap, src_ap)

            def qt_group(g):
                # 4 transfers of 5 chunks ([128, 1500] fp8)
                for h in range(4):
                    dma(
                        qt_sb[g][:, h * 5 * GQ : (h + 1) * 5 * GQ],
                        qtg_d[g, :, h * 5 * GQ : (h + 1) * 5 * GQ],
                    )

            stb_half(0)
            emit_consts()
            qt_group(0)
            stb_half(1)
            for g in range(1, G):
                qt_group(g)

            hd.update(stb=stb_tile, qt=qt_sb)
            return hd

        def emit_compute(hd):
            def early_out():
                zt = consts.tile([128, GQ], F32, tag="outsb")
                nc.vector.memset(zt, 0.0)
                nc.sync.dma_start(out=out_d[:, :], in_=zt[: G * 20, :])

            stb = hd["stb"]
            qt_sb = hd["qt"]
            maskq_sb, nine_sb, i2_sb, cib_sb = (
                hd["maskq"], hd["nine"], hd["i2"], hd["cib"])
            h2_sb, hmo_sb, scale_sb = hd["h2"], hd["hmo"], hd["scale"]
            d1_sb = hd["d1"]

            def stw(c, g):
                return stb[:, c * SW + g * 128 : c * SW + (g + 1) * 128]

            # compat psum tiles + emission helper: compat-g matmul blocks are
            # interleaved into PE gaps of the gram/NS/ADMM phases, tracking
            # the qt DMA stream (g0 between the stb halves, g1.. later).
            cpsum_ctx = ExitStack()
            cpsum = cpsum_ctx.enter_context(
                tc.tile_pool(name="cpsum", bufs=4, space="PSUM"))
            cp_all = {}
            cs_all = {}

            def compat_block(g, c0, c1):
                if upto < 4:
                    return
                if g not in cp_all:
                    cp = cpsum.tile([128, GQ], F32, tag="cp")
                    cp_all[g] = cp
                for c in range(c0, c1):
                    nc.tensor.matmul(
                        cp_all[g],
                        lhsT=stw(c, g),
                        rhs=qt_sb[g][:, c * GQ : (c + 1) * GQ],
                        start=(c == 0),
                        stop=(c == NCH - 1),
                    )

            def compat_copy(g):
                # PSUM -> SBUF bf16 on the (otherwise idle) Pool engine
                if upto < 4:
                    return
                cs = csb_pool.tile([128, GQ], BF16, tag="cs")
                nc.scalar.activation(
                    cs, cp_all[g], mybir.ActivationFunctionType.Copy
                )
                cs_all[g] = cs

            # ---- stage 1: K = S S^T per 4-task window, bf16 ---------------
            # chunk-outer so the matmuls track the stb DMA stream; 4 banks.
            # compat-g0 blocks ride the gaps (qt-g0 lands mid-gram).
            hb_all = []
            with tc.tile_pool(name="kpsum", bufs=4, space="PSUM") as kpsum:
                kp_all = []
                for g in range(G):
                    kp = kpsum.tile([128, 128], F32, tag="kp")
                    kp_all.append(kp)
                for c in range(10):
                    for g in range(G):
                        nc.tensor.matmul(
                            kp_all[g], lhsT=stw(c, g), rhs=stw(c, g),
                            start=(c == 0), stop=False,
                        )
                compat_block(0, 0, 5)
                compat_block(0, 5, 10)
                for c in range(10, NCH):
                    for g in range(G):
                        nc.tensor.matmul(
                            kp_all[g], lhsT=stw(c, g), rhs=stw(c, g),
                            start=False, stop=(c == NCH - 1),
                        )
                    if c % 2 == 1:
                        compat_block(0, c - 1, c + 1)
                for g in range(G):
                    km = mats.tile([128, 128], F32, tag="km")
                    nc.vector.tensor_tensor(km, kp_all[g], maskq_sb, op=AluOpType.mult)
                    hb = mats.tile([128, 128], BF16, tag="hb")
                    nc.vector.tensor_tensor(hb, km, nine_sb, op=AluOpType.add)
                    hb_all.append(hb)
            compat_copy(0)

            if upto < 2:
                cpsum_ctx.close()
                return early_out()
            # ---- stage 2: Newton-Schulz inverse, all-bf16 -----------------
            # t1p-all / x2p-all per iter so the DVE hop of group g overlaps
            # the matmuls of groups g+1.. ; compat-g1 blocks between iters.
            wt_sb = []
            with tc.tile_pool(name="npsum", bufs=2, space="PSUM") as npsum:
                x_cur = [cib_sb] * G
                for it in range(NS_ITERS):
                    last = it == NS_ITERS - 1
                    u_all = []
                    for g in range(G):
                        t1p = npsum.tile([128, 128], F32, tag="t1p")
                        nc.tensor.matmul(
                            t1p, lhsT=hb_all[g], rhs=x_cur[g], start=True, stop=True
                        )
                        u_ns = mats.tile([128, 128], BF16, tag="u_nsb")
                        nc.vector.tensor_tensor(u_ns, i2_sb, t1p, op=AluOpType.subtract)
                        u_all.append(u_ns)
                    for g in range(G):
                        x2p = npsum.tile([128, 128], F32, tag="x2p")
                        nc.tensor.matmul(
                            x2p, lhsT=x_cur[g], rhs=u_all[g], start=True, stop=True
                        )
                        if last:
                            wt = mats.tile([128, 128], BF16, tag="wt")
                            nc.vector.tensor_copy(wt, x2p)
                            wt_sb.append(wt)
                        else:
                            x_next = mats.tile([128, 128], BF16, tag="x_nsb")
                            nc.vector.tensor_copy(x_next, x2p)
                            x_cur[g] = x_next
                    compat_block(1, 5 * it, 5 * (it + 1))

            if upto < 3:
                cpsum_ctx.close()
                return early_out()
            # ---- stage 3: over-relaxed ADMM (compressed state) ------------
            #   z   = center(W (v - u + ohc));  zh = a*z + (1-a)*v
            #   r   = zh + u - h;  v' = h + (r - |r|)/2;  u' = relu(r)
            #   d1' = (h + ohc) - |r|  [consts pre-scaled by alpha*rho]
            # compat g1/g2 blocks fill the PE while each iter's DVE chain runs.
            xb_sb = None
            with tc.tile_pool(name="mpsum", bufs=2, space="PSUM") as mpsum:
                s_sb = hd["s0"]
                for it in range(ADMM_ITERS):
                    last = it == ADMM_ITERS - 1
                    xp = mpsum.tile([128, 20], F32, tag="mp")
                    for g in range(G):
                        nc.tensor.matmul(
                            xp[:, g * NW : (g + 1) * NW],
                            lhsT=wt_sb[g],
                            rhs=d1_sb[:, g * NW : (g + 1) * NW],
                            start=True,
                            stop=True,
                        )
                    # fill PE while this iter's DVE chain produces d1'
                    if it == 0:
                        compat_block(1, 10, 20)
                    elif it == 1:
                        compat_block(2, 0, 10)
                    elif it == 2:
                        compat_block(2, 10, 20)
                    else:
                        compat_block(3, 0, 10)
                    msum = state.tile([128, 4], F32, tag="msum")
                    nc.vector.reduce_sum(
                        msum,
                        xp[:, :].rearrange("p (g w) -> p g w", w=NW),
                        axis=mybir.AxisListType.X,
                    )
                    msb = msum[:, :]
                    msb_ap = bass.AP(
                        tensor=msb.tensor, offset=msb.offset,
                        ap=[msb.ap[0], msb.ap[1], [0, NW]],
                    )
                    zn_sb = state.tile([128, 20], BF16 if last else F32,
                                       tag="xb" if last else "zn")
                    nc.vector.scalar_tensor_tensor(
                        out=zn_sb[:, :].rearrange("p (g w) -> p g w", w=NW),
                        in0=msb_ap,
                        scalar=-1.0 / NW,
                        in1=xp[:, :].rearrange("p (g w) -> p g w", w=NW),
                        op0=AluOpType.mult,
                        op1=AluOpType.add,
                    )
                    if last:
                        xb_sb = zn_sb
                        break
                    r_sb = state.tile([128, 20], F32, tag="r")
                    nc.vector.tensor_tensor(r_sb, zn_sb, s_sb, op=AluOpType.subtract)
                    a_sb = state.tile([128, 20], F32, tag="absr")
                    nc.vector.scalar_tensor_tensor(
                        out=a_sb, in0=r_sb, scalar=-1.0, in1=r_sb,
                        op0=AluOpType.mult, op1=AluOpType.max,
                    )
                    d1_sb = state.tile([128, 20], BF16, tag="d1n")
                    nc.vector.scalar_tensor_tensor(
                        out=d1_sb, in0=a_sb, scalar=-(RHO * ALPHA), in1=h2_sb,
                        op0=AluOpType.mult, op1=AluOpType.add,
                    )
                    # off-critical-path state maintenance:
                    m1_sb = state.tile([128, 20], F32, tag="m1")
                    nc.vector.scalar_tensor_tensor(
                        out=m1_sb, in0=r_sb, scalar=(1.0 - ALPHA), in1=r_sb,
                        op0=AluOpType.mult, op1=AluOpType.max,
                    )
                    s_sb = state.tile([128, 20], F32, tag="sst")
                    nc.vector.tensor_tensor(s_sb, hmo_sb, m1_sb, op=AluOpType.subtract)
            compat_copy(1)
            compat_copy(2)

            # xdiag: block-diagonal solution tiles, scale folded in.
            # xdiag[tp*32+s, tp*5+w] = xb[tp*32+s, g*5+w] * scale
            xdiag_all = []
            for g in range(G):
                xdg = state.tile([128, 20], BF16, tag=f"xdiag{g}")
                nc.vector.memset(xdg, 0.0)
                xdiag_all.append(xdg)
            for g in range(G):
                for tp in range(GP):
                    sl = slice(tp * 32, tp * 32 + NS)
                    ssb = scale_sb[sl, :]
                    sc_ap = bass.AP(
                        tensor=ssb.tensor, offset=ssb.offset,
                        ap=[ssb.ap[0], [0, NW]],
                    )
                    eng = nc.vector if (g * GP + tp) % 2 == 0 else nc.gpsimd
                    eng.tensor_tensor(
                        xdiag_all[g][sl, tp * NW : (tp + 1) * NW],
                        xb_sb[sl, g * NW : (g + 1) * NW],
                        sc_ap,
                        op=AluOpType.mult,
                    )

            if upto < 4:
                cpsum_ctx.close()
                return early_out()
            # ---- finish compat g2/g3; per-group logits as soon as ready ---
            # logits_g = xdiag_g^T compat_g: col-tiled matmuls into one PSUM
            # bank; strips at partitions 32g..32g+20. Strip copies on Pool,
            # then strip DMAs to out rows 20g.
            with tc.tile_pool(name="lpsum", bufs=4, space="PSUM") as lpsum:
                out_sb = consts.tile([128, GQ], F32, tag="outsb")

                def logits_group(g):
                    if upto < 5:
                        return
                    lp = lpsum.tile([128, GQ], F32, tag="lp")
                    sl = slice(32 * g, 32 * g + 20)
                    nc.tensor.matmul(
                        lp[sl, :],
                        lhsT=xdiag_all[g],
                        rhs=cs_all[g],
                        start=True,
                        stop=True,
                        tile_position=(0, 32 * g),
                    )
                    if g % 2 == 0:
                        nc.scalar.activation(
                            out_sb[sl, :], lp[sl, :],
                            mybir.ActivationFunctionType.Copy,
                        )
                    else:
                        nc.vector.tensor_copy(out_sb[sl, :], lp[sl, :])
                    eng = nc.sync if g % 2 == 0 else nc.scalar
                    eng.dma_start(
                        out=out_d[g * 20 : (g + 1) * 20, :],
                        in_=out_sb[sl, :],
                    )

                compat_block(3, 10, 20)
                compat_copy(3)
                logits_group(0)
                logits_group(1)
                logits_group(2)
                logits_group(3)
            cpsum_ctx.close()
            if upto < 5:
                return early_out()
            if variant == "debug":
                nc.sync.dma_start(out=dbg_h[:, :], in_=hb_all[0])
                dwt = mats.tile([128, 128], F32, tag="dbgwt")
                nc.vector.tensor_copy(dwt, wt_sb[0])
                nc.sync.dma_start(out=dbg_wt[:, :], in_=dwt)
                nc.sync.dma_start(out=dbg_xb[:, :], in_=xb_sb)
                nc.sync.dma_start(out=dbg_cs[:, :], in_=cs_all[0])

        def emit_body():
            if variant == "dma_only":
                hd = emit_loads()
                zt = consts.tile([128, GQ], F32, tag="outsb")
                nc.vector.memset(zt, 0.0)
                nc.sync.dma_start(out=out_d[:, :], in_=zt[: G * 20, :])
            else:
                hd = emit_loads()
                emit_compute(hd)

        if variant == "compute_only":
            hd0 = emit_loads()
        if repeat > 1:
            try:
                ctx.enter_context(tc.For_i(0, repeat, 1, staggered_reset=True))
            except Exception:
                ctx.enter_context(tc.For_i(0, repeat, 1))
        for _ in range(unroll):
            if variant == "compute_only":
                emit_compute(hd0)
            else:
                emit_body()

    _split_waits(nc)
    return nc


_NC_CACHE = None


def _get_nc():
    global _NC_CACHE
    if _NC_CACHE is None:
        _NC_CACHE = _build_program()
    return _NC_CACHE


# ---------------------------------------------------------------------------
def _host_prep(support, query, support_labels, scale):
    """Shard + pack into the DMA layouts. Layout only, no FLOPs."""
    f32 = np.float32
    bf = mybir.dt.np(BF16)
    e3 = mybir.dt.np(QDT)
    eyebd = np.zeros((128, 128), dtype=f32)     # eye-25 block diagonal
    onesbd = np.zeros((128, 128), dtype=f32)    # ones 25x25 block diagonal
    for tp in range(GP):
        sl = slice(tp * 32, tp * 32 + NS)
        eyebd[sl, sl] = np.eye(NS, dtype=f32)
        onesbd[sl, sl] = 1.0
    maskq = np.ascontiguousarray(onesbd.astype(bf))
    nine = np.ascontiguousarray((1.0 + RHO) * eyebd)
    i2 = np.ascontiguousarray(2.0 * eyebd)
    cib = np.ascontiguousarray((NS_C * eyebd).astype(bf))
    sc = np.asarray(scale, dtype=f32).reshape(1, 1) / (QSCALE * ALPHA)

    in_maps = []
    for core in range(N_CORES):
        sl = slice(core * T, (core + 1) * T)
        S = np.asarray(support[sl], dtype=f32)        # [16,25,2560]
        Q = np.asarray(query[sl], dtype=f32)          # [16,75,2560]
        lab = np.asarray(support_labels[sl])          # [16,25] int
        # stb[p, c*512 + g*128 + tp*32 + s] = S[4g+tp, s, 128c+p]
        src = S.reshape(G, GP, NS, NCH, 128).transpose(4, 3, 0, 1, 2)
        arr = np.zeros((128, NCH, G, GP, 32), dtype=bf)
        arr[..., :NS] = src.astype(bf)
        stb = np.ascontiguousarray(arr.reshape(128, NCH * SW))
        # qtg[g, p, c*300 + tp*75 + q] = clip(64*Q)[4g+tp, q, 128c+p]
        q8 = np.clip(Q * QSCALE, -15.5, 15.5)
        qsrc = q8.reshape(G, GP, NQ, NCH, 128).transpose(0, 4, 3, 1, 2)
        qtg = np.ascontiguousarray(
            qsrc.reshape(G, 128, NCH * GQ).astype(e3)
        )
        oh = (lab[:, :, None] == np.arange(NW)[None, None, :]).astype(f32)
        # [16,25,5] -> [128,20]: row = tp*32+s, col = g*5+w
        ohm = np.zeros((128, 20), dtype=f32)
        ohr = oh.reshape(G, GP, NS, NW).transpose(1, 2, 0, 3).reshape(GP, NS, 20)
        for tp in range(GP):
            ohm[tp * 32 : tp * 32 + NS, :] = ohr[tp]
        in_maps.append(
            {
                "stb": stb,
                "qtg": qtg,
                "maskq": maskq,
                "nine": nine,
                "i2": i2,
                "cib": cib,
                "ohc": np.ascontiguousarray(ALPHA * ohm),
                "h2": np.ascontiguousarray(RHO * ALPHA * (C_REG + 1.0 / RHO) * ohm),
                "hmo": np.ascontiguousarray(ALPHA * C_REG * ohm),
                "scale": sc,
            }
        )
    return in_maps


def _unshard_out(o):
    """[80, GQ] -> [T, NQ, NW]. o[20g + 5tp + w, 75tp' + q] valid at tp==tp'."""
    r = np.asarray(o, np.float32).reshape(G, GP, NW, GP, NQ)
    d = np.einsum('gtwtq->gtqw', r)              # diagonal over tp
    return d.reshape(T, NQ, NW)


def kernel(query, support, scale, support_labels, n_way, n_shot):
    assert int(n_way) == NW and int(n_shot) * int(n_way) == NS
    assert query.shape == (B_TOT, NQ, D) and support.shape == (B_TOT, NS, D)
    nc = _get_nc()
    in_maps = _host_prep(support, query, support_labels, scale)
    res = run_bass_kernel_spmd(nc, in_maps, core_ids=list(range(N_CORES)))
    outs = []
    for core in range(N_CORES):
        o = np.asarray(res.results[core]["out"])      # [80, GQ]
        outs.append(_unshard_out(o))
    return np.ascontiguousarray(np.concatenate(outs, axis=0), dtype=np.float32)
